# revision 18
# baseline (speedup 1.0000x reference)
"""Conv2d-via-FFT reference implemented as a direct convolution on TRN2.

The reference pads to FFT size 61 >= 32+3-1, so its circular cross-correlation
equals the linear valid cross-correlation: out[n,f,i,j] =
sum_{c,p,q} x[n,c,i+p,j+q] * w[f,c,p,q] + bias[f].  That is an ordinary
stride-1 valid conv2d, which maps onto the PE array as 9 accumulated matmuls
(one per filter tap) with C=128 on the contraction partitions.

Design notes (v5):
  * float16 operands.  Matmul streaming is 1 col/cycle for fp32r and fp16
    alike (stream floor 450cols/2.4GHz = 187.5ns/MM; measured cadence 190ns),
    but fp32 LDWEIGHTS (227ns) serializes into the cadence (259ns) while
    fp16 LDWEIGHTS (~97ns, FWL) hides completely.  randn data is unit-scale
    so fp16 costs ~5e-4 rel err.  fp16 also halves DMA traffic.
  * Every dma_start has a ~3us issue->data-ready pipe (descriptor gen +
    SDMA fetch + transfer + completion receipt) and occupies its ring for
    ~2us, so the DMAs the stream start depends on are spread over FOUR
    channels: sync ring (x sample0 rows0:17), scalar ring (w taps 0-3),
    gpsimd SWDGE (w taps 4-8); everything else follows behind.
  * Chunk schedule: sample0-rows0:15 first, then samples 1-7 (2x15 rows),
    then sample0 rows 15:27 and 27:30 LAST - so its x rows 17:32 can ride
    late on the sync ring, and the final chunk is only 90 cols, putting the
    last output DMA ~0.8us after the last matmul (the ~3us completion
    receipt of that DMA is the tail floor).
  * Per-chunk drain alternates Scalar ACT / DVE tensor_scalar_add (both add
    bias, cast to fp16) into a dedicated per-sample [F,900] slot; one
    output DMA per sample (early samples on the scalar ring, late on sync).
  * 10 warmup matmuls on garbage keep the PE busy from queue start (~7.25us)
    until data lands (~11us) so the HAM clock gate sees continuous activity.

Sharding: data-parallel over N (64 samples -> 8 per core), filter replicated.
"""

import numpy as np

import concourse.bass as bass
import concourse.bacc as bacc
import concourse.mybir as mybir
from concourse.bass_utils import run_bass_kernel_spmd

dt = mybir.dt
F32 = dt.float32
F16 = dt.float16
IDENT = mybir.ActivationFunctionType.Identity

N, C, H, W = 64, 128, 32, 32
F, KH, KW = 128, 3, 3
KK = KH * KW
OH, OW = H - KH + 1, W - KW + 1          # 30, 30
NCORES = 8
NPC = N // NCORES                        # samples per core
NWARM = 11

# chunk schedule: (sample, row0, nrows); list order = execution order.
# chunk j accumulates its 9 tap-matmuls into PSUM bank j%8 and drains on
# Scalar (even j) or DVE (odd j) into o_sb[sample][:, row0*OW:...].
CHUNKS = ([(0, 0, 15)]
          + [(n, r, 15) for n in range(1, NPC) for r in (0, 15)]
          + [(0, 15, 14), (0, 29, 1)])
NCHUNK = len(CHUNKS)                     # 17

# drain ordinal (1-based) per chunk on its engine
_sc_ord, _ve_ord, _ords = 0, 0, []
for _j in range(NCHUNK):
    if _j % 2 == 0:
        _sc_ord += 1
        _ords.append(_sc_ord)
    else:
        _ve_ord += 1
        _ords.append(_ve_ord)


def _sample_done(n):
    """(s_sc threshold, s_ve threshold) after which sample n's slot is full."""
    sc = max([_ords[j] for j, (m, _, _) in enumerate(CHUNKS)
              if m == n and j % 2 == 0], default=0)
    ve = max([_ords[j] for j, (m, _, _) in enumerate(CHUNKS)
              if m == n and j % 2 == 1], default=0)
    return sc, ve


def _build():
    nc = bacc.Bacc("TRN2", target_bir_lowering=False, debug=False)

    x_d = nc.dram_tensor("x", [C, NPC, H, W], F16, kind="ExternalInput").ap()
    w_d = nc.dram_tensor("w", [C, KK, F], F16, kind="ExternalInput").ap()
    b_d = nc.dram_tensor("bias", [F, 1], F32, kind="ExternalInput").ap()
    o_d = nc.dram_tensor("out", [NPC, F, OH * OW], F16, kind="ExternalOutput").ap()

    w_sb = nc.alloc_sbuf_tensor("w_sb", [C, KK, F], F16).ap()
    b_sb = nc.alloc_sbuf_tensor("b_sb", [F, 1], F32).ap()
    x_sb = nc.alloc_sbuf_tensor("x_sb", [C, NPC, H, W], F16).ap()
    o_sb = [nc.alloc_sbuf_tensor(f"o_sb{n}", [F, OH * OW], F16).ap()
            for n in range(NPC)]
    ps = [nc.alloc_psum_tensor(f"ps{i}", [F, 15 * OW], F32).ap()
          for i in range(8)]

    # HWDGE semantics: a DMA's +16 arrives as 16 independent +1s (one per
    # SDMA engine), so every DMA gets its own sem, waited at multiples of 16.
    # Sem numbers pinned at 207+ so the NEFF epilogue's blanket per-engine
    # sem reset stays sound without an exit barrier.
    from contextlib import ExitStack
    with ExitStack() as ctx:
      _next_num = iter(range(207, 255))
      sem = lambda nm: ctx.enter_context(nc.semaphore(nm, num=next(_next_num)))
      s_wa = sem("s_wa")                  # w taps 0-3
      s_wb = sem("s_wb")                  # w taps 4-8
      s_b = sem("s_b")
      s_x0a = sem("s_x0a")                # sample 0 rows 0:17
      s_x0b = sem("s_x0b")                # sample 0 rows 17:32
      s_x = [sem(f"s_x{n}") for n in range(1, NPC)]
      s_mm = sem("s_mm")                  # chunk accumulations complete
      s_sc = sem("s_sc")                  # scalar drains done
      s_ve = sem("s_ve")                  # dve drains done
      s_o = [sem(f"s_o{n}") for n in range(NPC)]
      s_o0b = sem("s_o0b")                # sample 0 cols 870:900 store

      _orig_barrier = nc.all_engine_barrier
      nc.all_engine_barrier = lambda *a, **k: None
      with nc.Block(no_gpsimd_drain=True) as block:

        def out_dma(eng, n):
            sc, ve = _sample_done(n)
            eng.wait_ge(s_sc, sc)
            eng.wait_ge(s_ve, ve)
            eng.dma_start(o_d[n], o_sb[n]).then_inc(s_o[n], 16)

        @block.sync
        def _(sync):
            sync.dma_start(x_sb[:, 0, 0:17], x_d[:, 0, 0:17]).then_inc(s_x0a, 16)
            for n in range(1, NPC):
                sync.dma_start(x_sb[:, n], x_d[:, n]).then_inc(s_x[n - 1], 16)
            sync.dma_start(x_sb[:, 0, 17:32],
                           x_d[:, 0, 17:32]).then_inc(s_x0b, 16)
            for n in (6, 7):
                out_dma(sync, n)
            # sample 0 cols 0:870: ready once chunk 0 (scalar drain #1) and
            # the rows-15:29 chunk (DVE drain #8) have drained
            sync.wait_ge(s_sc, 1)
            sync.wait_ge(s_ve, _ords[15])
            sync.dma_start(o_d[0, :, 0:870], o_sb[0][:, 0:870]).then_inc(s_o[0], 16)
            for n in range(NPC):          # all outputs landed in DRAM
                sync.wait_ge(s_o[n], 16)
            sync.wait_ge(s_o0b, 16)

        @block.scalar
        def _(scalar):
            scalar.dma_start(w_sb[:, 0:4], w_d[:, 0:4]).then_inc(s_wa, 16)
            scalar.dma_start(b_sb[:], b_d[:]).then_inc(s_b, 16)
            first = True
            for j, (n, row0, nrows) in enumerate(CHUNKS):
                if j % 2:
                    continue
                px = nrows * OW
                if first:
                    scalar.wait_ge(s_b, 16)
                    first = False
                scalar.wait_ge(s_mm, j + 1)
                nc.scalar.activation(
                    o_sb[n][:, row0 * OW:row0 * OW + px],
                    ps[j % 8][:, :px], IDENT,
                    bias=b_sb[:]).then_inc(s_sc, 1)
                if 1 <= n <= 5 and row0 == 15:
                    out_dma(scalar, n)
                elif j == NCHUNK - 1:
                    # sample 0 cols 870:900 - the last 30 output pixels
                    scalar.dma_start(o_d[0, :, 870:900],
                                     o_sb[0][:, 870:900]).then_inc(s_o0b, 16)

        @block.vector
        def _(vector):
            first = True
            for j, (n, row0, nrows) in enumerate(CHUNKS):
                if j % 2 == 0:
                    continue
                px = nrows * OW
                if first:
                    vector.wait_ge(s_b, 16)
                    first = False
                vector.wait_ge(s_mm, j + 1)
                nc.vector.tensor_scalar_add(
                    o_sb[n][:, row0 * OW:row0 * OW + px],
                    ps[j % 8][:, :px], b_sb[:]).then_inc(s_ve, 1)

        @block.gpsimd
        def _(gpsimd):
            gpsimd.dma_start(w_sb[:, 4:9], w_d[:, 4:9]).then_inc(s_wb, 16)

        @block.tensor
        def _(tensor):
            # No-dependency warmup matmuls on whatever is in SBUF: the PE is
            # busy from queue start until real data lands, so the HAM clock
            # gate (needs ~3.4us of continuous activity) opens as early as
            # its free-running window allows.  Bank 7's first real use is
            # chunk index 7, ~12us after the warmups finish.
            for _ in range(NWARM):
                nc.tensor.matmul(ps[7][:], w_sb[:, 0], x_sb[:, 0, 0:15, 0:30],
                                 start=True, stop=True)
            seen_x = set()
            for j, (n, row0, nrows) in enumerate(CHUNKS):
                px = nrows * OW
                # first-matmul dependencies; all but the last are standalone
                # waits (1 sem wait per instruction).
                waits = []
                if j == 0:
                    waits += [(s_wa, 16), (s_x0a, 16)]
                elif n == 0 and row0 == 15:
                    waits.append((s_x0b, 16))
                elif n >= 1 and n not in seen_x:
                    waits.append((s_x[n - 1], 16))
                seen_x.add(n)
                if j >= 8:
                    # PSUM bank j%8 free once chunk j-8 drained
                    prev = j - 8
                    waits.append((s_sc if prev % 2 == 0 else s_ve,
                                  _ords[prev]))
                for s, v in waits[:-1]:
                    tensor.wait_ge(s, v)
                for k in range(KK):
                    p, q = divmod(k, KW)
                    mm = nc.tensor.matmul(
                        ps[j % 8][:, :px],
                        w_sb[:, k],
                        x_sb[:, n, row0 + p:row0 + p + nrows, q:q + OW],
                        start=(k == 0),
                        stop=(k == KK - 1),
                    )
                    if k == 0 and waits:
                        mm._wait_ge(*waits[-1])
                    elif j == 0 and k == 4:
                        mm._wait_ge(s_wb, 16)
                    if k == KK - 1:
                        mm.then_inc(s_mm, 1)

      nc.all_engine_barrier = _orig_barrier

    nc.compile()
    return nc


_NC = None


def _get_nc():
    global _NC
    if _NC is None:
        _NC = _build()
    return _NC


def _in_maps(x, w, bias):
    w_prep = np.ascontiguousarray(
        w.transpose(1, 2, 3, 0).reshape(C, KK, F).astype(np.float16))
    b_prep = np.ascontiguousarray(bias.astype(np.float32).reshape(F, 1))
    maps = []
    for c in range(NCORES):
        xc = np.ascontiguousarray(
            x[c * NPC:(c + 1) * NPC].transpose(1, 0, 2, 3).astype(np.float16))
        maps.append({"x": xc, "w": w_prep, "bias": b_prep})
    return maps


def run(x, w, bias, trace=False, **spmd_kwargs):
    """Run the SPMD kernel; returns (out [N,F,OH,OW], BassKernelResults)."""
    nc = _get_nc()
    res = run_bass_kernel_spmd(nc, _in_maps(x, w, bias), list(range(NCORES)),
                               trace=trace, **spmd_kwargs)
    parts = [res.results[c]["out"].astype(np.float32).reshape(NPC, F, OH, OW)
             for c in range(NCORES)]
    return np.concatenate(parts, axis=0), res


def kernel(x, w, bias):
    out, _ = run(np.asarray(x), np.asarray(w), np.asarray(bias))
    return out


# revision 20
# speedup vs baseline: 1.0121x; 1.0121x over previous
"""Conv2d-via-FFT reference implemented as a direct convolution on TRN2.

The reference pads to FFT size 61 >= 32+3-1, so its circular cross-correlation
equals the linear valid cross-correlation: out[n,f,i,j] =
sum_{c,p,q} x[n,c,i+p,j+q] * w[f,c,p,q] + bias[f].  That is an ordinary
stride-1 valid conv2d, which maps onto the PE array as 9 accumulated matmuls
(one per filter tap) with C=128 on the contraction partitions.

Design notes:
  * float16 operands.  Matmul streaming is 1 col/cycle for fp32r and fp16
    alike (stream floor 450cols/2.4GHz = 187.5ns/MM; measured cadence 190ns),
    but fp32 LDWEIGHTS (227ns) serializes into the cadence (259ns) while
    fp16 LDWEIGHTS (~97ns, FWL) hides completely.  randn data is unit-scale
    so fp16 costs ~5e-4 rel err.  fp16 also halves DMA traffic.
  * Every dma_start has a ~3us issue->data-ready pipe (descriptor gen +
    SDMA fetch + transfer + completion receipt) and occupies its ring for
    ~2us, so the DMAs the stream start depends on are spread over FOUR
    channels: sync ring (x sample0 rows0:17), scalar ring (w taps 0-3),
    gpsimd SWDGE (w taps 4-8); everything else follows behind.
  * Chunk schedule: sample0-rows0:15 first, then samples 1-7 (2x15 rows),
    then sample0 rows 15:29 and 29:30 LAST - so its x rows 17:32 can ride
    late on the sync ring, and the final chunk is only 30 cols, putting the
    last output DMA issue ~0.9us after the last matmul (the ~3us completion
    receipt of that DMA is the tail floor).  Sample 0's store is split
    [0:870] (sync ring, after the rows-15:29 DVE drain) / [870:900]
    (scalar ring, right after the final 1-row ACT) so the two final DMAs'
    receipts overlap.
  * Per-chunk drain alternates Scalar ACT / DVE tensor_scalar_add (both add
    bias, cast to fp16) into a dedicated per-sample [F,900] slot; one
    output DMA per sample (early samples on the scalar ring, late on sync).
  * 11 warmup matmuls on garbage keep the PE busy from queue start (~7.25us)
    until data lands (~11.4us) so the HAM clock gate sees continuous
    activity and opens at the earliest window its free-running phase allows.

Sharding: data-parallel over N (64 samples -> 8 per core), filter replicated.
"""

import numpy as np

import concourse.bass as bass
import concourse.bacc as bacc
import concourse.mybir as mybir
from concourse.bass_utils import run_bass_kernel_spmd

dt = mybir.dt
F32 = dt.float32
F16 = dt.float16
IDENT = mybir.ActivationFunctionType.Identity

N, C, H, W = 64, 128, 32, 32
F, KH, KW = 128, 3, 3
KK = KH * KW
OH, OW = H - KH + 1, W - KW + 1          # 30, 30
NCORES = 8
NPC = N // NCORES                        # samples per core
NWARM = 11

# chunk schedule: (sample, row0, nrows); list order = execution order.
# chunk j accumulates its 9 tap-matmuls into PSUM bank j%8 and drains on
# Scalar (even j) or DVE (odd j) into o_sb[sample][:, row0*OW:...].
CHUNKS = ([(0, 0, 15)]
          + [(n, r, 15) for n in range(1, NPC) for r in (0, 15)]
          + [(0, 15, 14), (0, 29, 1)])
NCHUNK = len(CHUNKS)                     # 17

# drain ordinal (1-based) per chunk on its engine
_sc_ord, _ve_ord, _ords = 0, 0, []
for _j in range(NCHUNK):
    if _j % 2 == 0:
        _sc_ord += 1
        _ords.append(_sc_ord)
    else:
        _ve_ord += 1
        _ords.append(_ve_ord)


def _sample_done(n):
    """(s_sc threshold, s_ve threshold) after which sample n's slot is full."""
    sc = max([_ords[j] for j, (m, _, _) in enumerate(CHUNKS)
              if m == n and j % 2 == 0], default=0)
    ve = max([_ords[j] for j, (m, _, _) in enumerate(CHUNKS)
              if m == n and j % 2 == 1], default=0)
    return sc, ve


def _build():
    nc = bacc.Bacc("TRN2", target_bir_lowering=False, debug=False)

    x_d = nc.dram_tensor("x", [C, NPC, H, W], F16, kind="ExternalInput").ap()
    w_d = nc.dram_tensor("w", [C, KK, F], F16, kind="ExternalInput").ap()
    b_d = nc.dram_tensor("bias", [F, 1], F32, kind="ExternalInput").ap()
    o_d = nc.dram_tensor("out", [NPC, F, OH * OW], F16, kind="ExternalOutput").ap()

    w_sb = nc.alloc_sbuf_tensor("w_sb", [C, KK, F], F16).ap()
    b_sb = nc.alloc_sbuf_tensor("b_sb", [F, 1], F32).ap()
    x_sb = nc.alloc_sbuf_tensor("x_sb", [C, NPC, H, W], F16).ap()
    o_sb = [nc.alloc_sbuf_tensor(f"o_sb{n}", [F, OH * OW], F16).ap()
            for n in range(NPC)]
    ps = [nc.alloc_psum_tensor(f"ps{i}", [F, 15 * OW], F32).ap()
          for i in range(8)]

    # HWDGE semantics: a DMA's +16 arrives as 16 independent +1s (one per
    # SDMA engine), so every DMA gets its own sem, waited at multiples of 16.
    # Sem numbers pinned at 207+ so the NEFF epilogue's blanket per-engine
    # sem reset stays sound without an exit barrier.
    from contextlib import ExitStack
    with ExitStack() as ctx:
      _next_num = iter(range(207, 255))
      sem = lambda nm: ctx.enter_context(nc.semaphore(nm, num=next(_next_num)))
      s_wa = sem("s_wa")                  # w taps 0-3
      s_wb = sem("s_wb")                  # w taps 4-8
      s_b = sem("s_b")
      s_x0a = sem("s_x0a")                # sample 0 rows 0:17
      s_x0b = sem("s_x0b")                # sample 0 rows 17:32
      s_x = [sem(f"s_x{n}") for n in range(1, NPC)]
      s_mm = sem("s_mm")                  # chunk accumulations complete
      s_sc = sem("s_sc")                  # scalar drains done
      s_ve = sem("s_ve")                  # dve drains done
      s_o = [sem(f"s_o{n}") for n in range(NPC)]
      s_o0b = sem("s_o0b")                # sample 0 cols 870:900 store

      _orig_barrier = nc.all_engine_barrier
      nc.all_engine_barrier = lambda *a, **k: None
      with nc.Block(no_gpsimd_drain=True) as block:

        def out_dma(eng, n):
            sc, ve = _sample_done(n)
            eng.wait_ge(s_sc, sc)
            eng.wait_ge(s_ve, ve)
            eng.dma_start(o_d[n], o_sb[n]).then_inc(s_o[n], 16)

        @block.sync
        def _(sync):
            sync.dma_start(x_sb[:, 0, 0:17], x_d[:, 0, 0:17]).then_inc(s_x0a, 16)
            for n in range(1, NPC):
                sync.dma_start(x_sb[:, n], x_d[:, n]).then_inc(s_x[n - 1], 16)
            sync.dma_start(x_sb[:, 0, 17:32],
                           x_d[:, 0, 17:32]).then_inc(s_x0b, 16)
            for n in (6, 7):
                out_dma(sync, n)
            # sample 0 cols 0:870: ready once chunk 0 (scalar drain #1) and
            # the rows-15:29 chunk (DVE drain #8) have drained
            sync.wait_ge(s_sc, 1)
            sync.wait_ge(s_ve, _ords[15])
            sync.dma_start(o_d[0, :, 0:870], o_sb[0][:, 0:870]).then_inc(s_o[0], 16)
            for n in range(NPC):          # all outputs landed in DRAM
                sync.wait_ge(s_o[n], 16)
            sync.wait_ge(s_o0b, 16)

        @block.scalar
        def _(scalar):
            scalar.dma_start(w_sb[:, 0:4], w_d[:, 0:4]).then_inc(s_wa, 16)
            scalar.dma_start(b_sb[:], b_d[:]).then_inc(s_b, 16)
            first = True
            for j, (n, row0, nrows) in enumerate(CHUNKS):
                if j % 2:
                    continue
                px = nrows * OW
                if first:
                    scalar.wait_ge(s_b, 16)
                    first = False
                scalar.wait_ge(s_mm, j + 1)
                nc.scalar.activation(
                    o_sb[n][:, row0 * OW:row0 * OW + px],
                    ps[j % 8][:, :px], IDENT,
                    bias=b_sb[:]).then_inc(s_sc, 1)
                if 1 <= n <= 5 and row0 == 15:
                    out_dma(scalar, n)
                elif j == NCHUNK - 1:
                    # sample 0 cols 870:900 - the last 30 output pixels
                    scalar.dma_start(o_d[0, :, 870:900],
                                     o_sb[0][:, 870:900]).then_inc(s_o0b, 16)

        @block.vector
        def _(vector):
            first = True
            for j, (n, row0, nrows) in enumerate(CHUNKS):
                if j % 2 == 0:
                    continue
                px = nrows * OW
                if first:
                    vector.wait_ge(s_b, 16)
                    first = False
                vector.wait_ge(s_mm, j + 1)
                nc.vector.tensor_scalar_add(
                    o_sb[n][:, row0 * OW:row0 * OW + px],
                    ps[j % 8][:, :px], b_sb[:]).then_inc(s_ve, 1)

        @block.gpsimd
        def _(gpsimd):
            gpsimd.dma_start(w_sb[:, 4:9], w_d[:, 4:9]).then_inc(s_wb, 16)

        @block.tensor
        def _(tensor):
            # No-dependency warmup matmuls on whatever is in SBUF: the PE is
            # busy from queue start until real data lands, so the HAM clock
            # gate (needs ~3.4us of continuous activity) opens as early as
            # its free-running window allows.  Bank 7's first real use is
            # chunk index 7, ~12us after the warmups finish.
            for _ in range(NWARM):
                nc.tensor.matmul(ps[7][:], w_sb[:, 0], x_sb[:, 0, 0:15, 0:30],
                                 start=True, stop=True)
            seen_x = set()
            for j, (n, row0, nrows) in enumerate(CHUNKS):
                px = nrows * OW
                # first-matmul dependencies; all but the last are standalone
                # waits (1 sem wait per instruction).
                waits = []
                if j == 0:
                    waits += [(s_wa, 16), (s_x0a, 16)]
                elif n == 0 and row0 == 15:
                    waits.append((s_x0b, 16))
                elif n >= 1 and n not in seen_x:
                    waits.append((s_x[n - 1], 16))
                seen_x.add(n)
                if j >= 8:
                    # PSUM bank j%8 free once chunk j-8 drained
                    prev = j - 8
                    waits.append((s_sc if prev % 2 == 0 else s_ve,
                                  _ords[prev]))
                for s, v in waits[:-1]:
                    tensor.wait_ge(s, v)
                for k in range(KK):
                    p, q = divmod(k, KW)
                    mm = nc.tensor.matmul(
                        ps[j % 8][:, :px],
                        w_sb[:, k],
                        x_sb[:, n, row0 + p:row0 + p + nrows, q:q + OW],
                        start=(k == 0),
                        stop=(k == KK - 1),
                    )
                    if k == 0 and waits:
                        mm._wait_ge(*waits[-1])
                    elif j == 0 and k == 4:
                        mm._wait_ge(s_wb, 16)
                    if k == KK - 1:
                        mm.then_inc(s_mm, 1)

      nc.all_engine_barrier = _orig_barrier

    nc.compile()
    return nc


_NC = None


def _get_nc():
    global _NC
    if _NC is None:
        _NC = _build()
    return _NC


def _in_maps(x, w, bias):
    w_prep = np.ascontiguousarray(
        w.transpose(1, 2, 3, 0).reshape(C, KK, F).astype(np.float16))
    b_prep = np.ascontiguousarray(bias.astype(np.float32).reshape(F, 1))
    maps = []
    for c in range(NCORES):
        xc = np.ascontiguousarray(
            x[c * NPC:(c + 1) * NPC].transpose(1, 0, 2, 3).astype(np.float16))
        maps.append({"x": xc, "w": w_prep, "bias": b_prep})
    return maps


def run(x, w, bias, trace=False, **spmd_kwargs):
    """Run the SPMD kernel; returns (out [N,F,OH,OW], BassKernelResults)."""
    nc = _get_nc()
    res = run_bass_kernel_spmd(nc, _in_maps(x, w, bias), list(range(NCORES)),
                               trace=trace, **spmd_kwargs)
    parts = [res.results[c]["out"].astype(np.float32).reshape(NPC, F, OH, OW)
             for c in range(NCORES)]
    return np.concatenate(parts, axis=0), res


def kernel(x, w, bias):
    out, _ = run(np.asarray(x), np.asarray(w), np.asarray(bias))
    return out


# revision 22
# speedup vs baseline: 1.0279x; 1.0155x over previous
"""Conv2d-via-FFT reference implemented as a direct convolution on TRN2.

The reference pads to FFT size 61 >= 32+3-1, so its circular cross-correlation
equals the linear valid cross-correlation: out[n,f,i,j] =
sum_{c,p,q} x[n,c,i+p,j+q] * w[f,c,p,q] + bias[f].  That is an ordinary
stride-1 valid conv2d, which maps onto the PE array as 9 accumulated matmuls
(one per filter tap) with C=128 on the contraction partitions.

Design notes:
  * float16 operands.  Matmul streaming is 1 col/cycle for fp32r and fp16
    alike (stream floor 450cols/2.4GHz = 187.5ns/MM; measured cadence 190ns),
    but fp32 LDWEIGHTS (227ns) serializes into the cadence (259ns) while
    fp16 LDWEIGHTS (~97ns, FWL) hides completely.  randn data is unit-scale
    so fp16 costs ~5e-4 rel err.  fp16 also halves DMA traffic.
  * Every dma_start has a ~3us issue->data-ready pipe (descriptor gen +
    SDMA fetch + transfer + completion receipt) and occupies its ring for
    ~2us, so the DMAs the stream start depends on are spread over FOUR
    channels: sync ring (x sample0 rows0:17), scalar ring (w taps 0-3),
    gpsimd SWDGE (w taps 4-8); everything else follows behind.
  * Chunk schedule: sample0-rows0:15 first, then samples 1-7 (2x15 rows),
    then sample0 rows 15:29 and 29:30 LAST - so its x rows 17:32 can ride
    late on the sync ring, and the final chunk is only 30 cols, putting the
    last output DMA issue ~0.9us after the last matmul (the ~3us completion
    receipt of that DMA is the tail floor).  Sample 0's store is split
    [0:870] (sync ring, after the rows-15:29 DVE drain) / [870:900]
    (scalar ring, right after the final 1-row ACT) so the two final DMAs'
    receipts overlap.
  * Per-chunk drain alternates Scalar ACT / DVE tensor_scalar_add (both add
    bias, cast to fp16) into a dedicated per-sample [F,900] slot; one
    output DMA per sample (early samples on the scalar ring, late on sync).
  * 11 warmup matmuls on garbage keep the PE busy from queue start (~7.25us)
    until data lands (~11.4us) so the HAM clock gate sees continuous
    activity and opens at the earliest window its free-running phase allows.

Sharding: data-parallel over N (64 samples -> 8 per core), filter replicated.
"""

import numpy as np

import concourse.bass as bass
import concourse.bacc as bacc
import concourse.mybir as mybir
from concourse.bass_utils import run_bass_kernel_spmd

dt = mybir.dt
F32 = dt.float32
F16 = dt.float16
IDENT = mybir.ActivationFunctionType.Identity

N, C, H, W = 64, 128, 32, 32
F, KH, KW = 128, 3, 3
KK = KH * KW
OH, OW = H - KH + 1, W - KW + 1          # 30, 30
NCORES = 8
NPC = N // NCORES                        # samples per core
NWARM = 9

# chunk schedule: (sample, row0, nrows); list order = execution order.
# chunk j accumulates its 9 tap-matmuls into PSUM bank j%8 and drains on
# Scalar (even j) or DVE (odd j) into o_sb[sample][:, row0*OW:...].
CHUNKS = ([(0, 0, 15)]
          + [(n, r, 15) for n in range(1, NPC) for r in (0, 15)]
          + [(0, 15, 14), (0, 29, 1)])
NCHUNK = len(CHUNKS)                     # 17

# drain ordinal (1-based) per chunk on its engine
_sc_ord, _ve_ord, _ords = 0, 0, []
for _j in range(NCHUNK):
    if _j % 2 == 0:
        _sc_ord += 1
        _ords.append(_sc_ord)
    else:
        _ve_ord += 1
        _ords.append(_ve_ord)


def _sample_done(n):
    """(s_sc threshold, s_ve threshold) after which sample n's slot is full."""
    sc = max([_ords[j] for j, (m, _, _) in enumerate(CHUNKS)
              if m == n and j % 2 == 0], default=0)
    ve = max([_ords[j] for j, (m, _, _) in enumerate(CHUNKS)
              if m == n and j % 2 == 1], default=0)
    return sc, ve


def _build():
    nc = bacc.Bacc("TRN2", target_bir_lowering=False, debug=False)

    x_d = nc.dram_tensor("x", [C, NPC, H, W], F16, kind="ExternalInput").ap()
    # taps 0-3 at cols 0:512, bias at col 512 (127 pad cols), taps 4-8 at
    # cols 640:1280 - so both w transfers start 4B-aligned and the bias
    # rides the first one (no separate bias DMA competing on the ring)
    w_d = nc.dram_tensor("w", [C, 1280], F16, kind="ExternalInput").ap()
    o_d = nc.dram_tensor("out", [NPC, F, OH * OW], F16, kind="ExternalOutput").ap()

    w_sb = nc.alloc_sbuf_tensor("w_sb", [C, 1280], F16).ap()
    b_sb = nc.alloc_sbuf_tensor("b_sb", [F, 1], F32).ap()
    x_sb = nc.alloc_sbuf_tensor("x_sb", [C, NPC, H, W], F16).ap()
    o_sb = [nc.alloc_sbuf_tensor(f"o_sb{n}", [F, OH * OW], F16).ap()
            for n in range(NPC)]
    ps = [nc.alloc_psum_tensor(f"ps{i}", [F, 15 * OW], F32).ap()
          for i in range(8)]

    # HWDGE semantics: a DMA's +16 arrives as 16 independent +1s (one per
    # SDMA engine), so every DMA gets its own sem, waited at multiples of 16.
    # Sem numbers pinned at 207+ so the NEFF epilogue's blanket per-engine
    # sem reset stays sound without an exit barrier.
    from contextlib import ExitStack
    with ExitStack() as ctx:
      _next_num = iter(range(207, 255))
      sem = lambda nm: ctx.enter_context(nc.semaphore(nm, num=next(_next_num)))
      s_wa = sem("s_wa")                  # w taps 0-3
      s_wb = sem("s_wb")                  # w taps 4-8
      s_x0a = sem("s_x0a")                # sample 0 rows 0:17
      s_x0b = sem("s_x0b")                # sample 0 rows 17:32
      s_x = [sem(f"s_x{n}") for n in range(1, NPC)]
      s_bc = sem("s_bc")                  # bias converted to fp32
      s_mm = sem("s_mm")                  # chunk accumulations complete
      s_sc = sem("s_sc")                  # scalar drains done
      s_ve = sem("s_ve")                  # dve drains done
      s_o = [sem(f"s_o{n}") for n in range(NPC)]
      s_o0b = sem("s_o0b")                # sample 0 cols 870:900 store

      _orig_barrier = nc.all_engine_barrier
      nc.all_engine_barrier = lambda *a, **k: None
      with nc.Block(no_gpsimd_drain=True) as block:

        def out_dma(eng, n):
            sc, ve = _sample_done(n)
            eng.wait_ge(s_sc, sc)
            eng.wait_ge(s_ve, ve)
            eng.dma_start(o_d[n], o_sb[n]).then_inc(s_o[n], 16)

        @block.sync
        def _(sync):
            sync.dma_start(x_sb[:, 0, 0:17], x_d[:, 0, 0:17]).then_inc(s_x0a, 16)
            for n in range(1, NPC):
                sync.dma_start(x_sb[:, n], x_d[:, n]).then_inc(s_x[n - 1], 16)
            sync.dma_start(x_sb[:, 0, 17:32],
                           x_d[:, 0, 17:32]).then_inc(s_x0b, 16)
            for n in (6, 7):
                out_dma(sync, n)
            # sample 0 cols 0:870: ready once chunk 0 (scalar drain #1) and
            # the rows-15:29 chunk (DVE drain #8) have drained
            sync.wait_ge(s_sc, 1)
            sync.wait_ge(s_ve, _ords[15])
            sync.dma_start(o_d[0, :, 0:870], o_sb[0][:, 0:870]).then_inc(s_o[0], 16)
            for n in range(NPC):          # all outputs landed in DRAM
                sync.wait_ge(s_o[n], 16)
            sync.wait_ge(s_o0b, 16)

        @block.scalar
        def _(scalar):
            scalar.dma_start(w_sb[:, 0:640], w_d[:, 0:640]).then_inc(s_wa, 16)
            first = True
            for j, (n, row0, nrows) in enumerate(CHUNKS):
                if j % 2:
                    continue
                px = nrows * OW
                if first:
                    scalar.wait_ge(s_bc, 1)
                    first = False
                scalar.wait_ge(s_mm, j + 1)
                nc.scalar.activation(
                    o_sb[n][:, row0 * OW:row0 * OW + px],
                    ps[j % 8][:, :px], IDENT,
                    bias=b_sb[:]).then_inc(s_sc, 1)
                if 1 <= n <= 5 and row0 == 15:
                    out_dma(scalar, n)
                elif j == NCHUNK - 1:
                    # sample 0 cols 870:900 - the last 30 output pixels
                    scalar.dma_start(o_d[0, :, 870:900],
                                     o_sb[0][:, 870:900]).then_inc(s_o0b, 16)

        @block.vector
        def _(vector):
            vector.wait_ge(s_wa, 16)
            nc.vector.tensor_copy(b_sb[:], w_sb[:, 512:513]).then_inc(s_bc, 1)
            first = False
            for j, (n, row0, nrows) in enumerate(CHUNKS):
                if j % 2 == 0:
                    continue
                px = nrows * OW
                vector.wait_ge(s_mm, j + 1)
                nc.vector.tensor_scalar_add(
                    o_sb[n][:, row0 * OW:row0 * OW + px],
                    ps[j % 8][:, :px], b_sb[:]).then_inc(s_ve, 1)

        @block.gpsimd
        def _(gpsimd):
            gpsimd.dma_start(w_sb[:, 640:1280],
                             w_d[:, 640:1280]).then_inc(s_wb, 16)

        @block.tensor
        def _(tensor):
            # No-dependency warmup matmuls on whatever is in SBUF: the PE is
            # busy from queue start until real data lands, so the HAM clock
            # gate (needs ~3.4us of continuous activity) opens as early as
            # its free-running window allows.  Bank 7's first real use is
            # chunk index 7, ~12us after the warmups finish.
            for _ in range(NWARM):
                nc.tensor.matmul(ps[7][:], w_sb[:, 0:F], x_sb[:, 0, 0:15, 0:30],
                                 start=True, stop=True)
            # paced warmups: the 16 per-SDMA-engine sem increments of a
            # DMA arrive spread over ~1.2us, so gating the last warmups
            # on partial counts keeps the PE busy right up to data-ready
            # even when the DMAs jitter late (protects the HAM gate).
            tensor.wait_ge(s_x0a, 16)
            nc.tensor.matmul(ps[7][:], w_sb[:, 0:F], x_sb[:, 0, 0:15, 0:30],
                             start=True, stop=True)
            tensor.wait_ge(s_wa, 12)
            nc.tensor.matmul(ps[7][:, 0:120], w_sb[:, 0:F],
                             x_sb[:, 0, 0:4, 0:30], start=True, stop=True)
            seen_x = set()
            for j, (n, row0, nrows) in enumerate(CHUNKS):
                px = nrows * OW
                # first-matmul dependencies; all but the last are standalone
                # waits (1 sem wait per instruction).
                waits = []
                if j == 0:
                    waits.append((s_wa, 16))
                elif n == 0 and row0 == 15:
                    waits.append((s_x0b, 16))
                elif n >= 1 and n not in seen_x:
                    waits.append((s_x[n - 1], 16))
                seen_x.add(n)
                if j >= 8:
                    # PSUM bank j%8 free once chunk j-8 drained
                    prev = j - 8
                    waits.append((s_sc if prev % 2 == 0 else s_ve,
                                  _ords[prev]))
                for s, v in waits[:-1]:
                    tensor.wait_ge(s, v)
                for k in range(KK):
                    p, q = divmod(k, KW)
                    wc = 128 * k if k < 4 else 640 + 128 * (k - 4)
                    mm = nc.tensor.matmul(
                        ps[j % 8][:, :px],
                        w_sb[:, wc:wc + F],
                        x_sb[:, n, row0 + p:row0 + p + nrows, q:q + OW],
                        start=(k == 0),
                        stop=(k == KK - 1),
                    )
                    if k == 0 and waits:
                        mm._wait_ge(*waits[-1])
                    elif j == 0 and k == 4:
                        mm._wait_ge(s_wb, 16)
                    if k == KK - 1:
                        mm.then_inc(s_mm, 1)

      nc.all_engine_barrier = _orig_barrier

    nc.compile()
    return nc


_NC = None


def _get_nc():
    global _NC
    if _NC is None:
        _NC = _build()
    return _NC


def _in_maps(x, w, bias):
    wt = w.transpose(1, 2, 3, 0).reshape(C, KK * F).astype(np.float16)
    w_prep = np.zeros((C, 1280), np.float16)
    w_prep[:, 0:512] = wt[:, 0:512]          # taps 0-3
    w_prep[:, 512] = bias.astype(np.float16)  # bias (partition = F index)
    w_prep[:, 640:1280] = wt[:, 512:1152]    # taps 4-8
    maps = []
    for c in range(NCORES):
        xc = np.ascontiguousarray(
            x[c * NPC:(c + 1) * NPC].transpose(1, 0, 2, 3).astype(np.float16))
        maps.append({"x": xc, "w": w_prep})
    return maps


def run(x, w, bias, trace=False, **spmd_kwargs):
    """Run the SPMD kernel; returns (out [N,F,OH,OW], BassKernelResults)."""
    nc = _get_nc()
    res = run_bass_kernel_spmd(nc, _in_maps(x, w, bias), list(range(NCORES)),
                               trace=trace, **spmd_kwargs)
    parts = [res.results[c]["out"].astype(np.float32).reshape(NPC, F, OH, OW)
             for c in range(NCORES)]
    return np.concatenate(parts, axis=0), res


def kernel(x, w, bias):
    out, _ = run(np.asarray(x), np.asarray(w), np.asarray(bias))
    return out
